# revision 22
# baseline (speedup 1.0000x reference)
"""Causal single-head attention on 8 Trainium2 NeuronCores — fully local.

Problem: x[4096,1024] -> Q,K,V = x@W.T+b (d_k=64), out = softmax(causal(QK^T/8)) @ V.

Strategy (replicated K/V, zero communication):
  - Every core loads the FULL x^T in bf16 (8 MB) and computes K^T and V for
    all 4096 rows locally; no collective, no cross-core sync of any kind.
    The 8 MB stream overlaps the projection/attention pipeline.
  - Query blocks of 128 rows; core c owns global blocks {c, 8+c, 16+c, 24+c}
    (strided) -> every core runs the IDENTICAL program. Slot j attends key
    blocks 0..8j+7 (uniform); within the diagonal band (blocks 8j..8j+7) a
    per-core host-built mask (ones/tri/zeros by key index vs c) enforces
    exact causality. Off-band blocks are always fully valid -> no masking.
  - x-column chunks of 512 stream in; chunk g yields key blocks 4g..4g+3.
    K^T/V^T come from a stacked [Wk|Wv] projection run as TWO interleaved
    256-column PSUM chains (keeps the PE p-state ramp hot); V~ blocks are PE
    transposes of V^T with a ones-column appended so the AV matmul also
    accumulates the softmax denominator.
  - Attention (scores -> exp -> mask -> AV) for every (q-slot, key-group)
    pair runs as soon as its chunk is projected; AV matmuls are emitted
    round-robin across slots so accumulation chains interleave on the PE.
  - All attention matmuls in bf16 (rate-1 at any moving width); exp on
    ScalarE with the 1/8 scale folded in; accumulation in f32 PSUM.
  - Constants ride in packed blobs to amortize per-DMA overhead; the x
    stream is ordered chunk0 -> xq -> chunks 1-7 so the projection pipeline
    starts as early as possible.

  PSUM discipline (hardware-verified): a PSUM bank supports ONE open
  accumulation group at a time. Interleaving two start/stop chains in one
  bank silently corrupts accumulation on hardware (the simulator does not
  model this). Hence: the kv projection's two interleaved column chains
  share a single group (start on the very first matmul, stop on the last,
  per-element has_written handles overwrite-vs-accumulate); each (chunk,
  slot) AV block runs as a CLOSED group (start..stop) in a rotating scratch
  bank and is accumulated into per-slot SBUF tiles on VectorE.
"""

import os
import numpy as np
import ml_dtypes
from contextlib import ExitStack

S, DM, DK = 4096, 1024, 64
NCORES = 8
QB = 128                      # rows per block
SLOTS = 4                     # q-blocks per core
SH = QB * SLOTS               # 512 own query rows per core
NB = S // QB                  # 32 key blocks
CHUNK = 512                   # x columns per streamed chunk
NCH = S // CHUNK              # 8 chunks
ND = DM // 128                # 8 contraction chunks

# cb_w bf16 blob (early): wkv [8, 128] | ident [128]
BF_WKV = 0
BF_ID = ND * 128
BFW_COLS = BF_ID + 128
# cb_r bf16 blob (later): wq [8, 64] | mask [8, 128]
BFR_WQ = 0
BFR_MASK = ND * DK
BFR_COLS = BFR_MASK + NCORES * QB
# f32 blob layout: bkv [1] | bq [1] | identf [128]
F_COLS = 2 + 128

AMP = int(os.environ.get("KERNEL_AMP", "1"))  # repeat whole pipeline in-NEFF

LAST_EXEC_NS = None


def _build_nc():
    import concourse.bass as bass
    import concourse.bacc as bacc
    import concourse.mybir as mybir
    import concourse.tile as tile

    f32 = mybir.dt.float32
    bf16 = mybir.dt.bfloat16
    AF = mybir.ActivationFunctionType

    nc = bacc.Bacc(None, num_devices=NCORES)

    xT_d = nc.dram_tensor("xT", [DM, S], bf16, kind="ExternalInput")
    xqT_d = nc.dram_tensor("xqT", [DM, SH], bf16, kind="ExternalInput")
    cbw_d = nc.dram_tensor("cbw", [128, BFW_COLS], bf16, kind="ExternalInput")
    cbr_d = nc.dram_tensor("cbr", [128, BFR_COLS], bf16, kind="ExternalInput")
    cf_d = nc.dram_tensor("cf", [128, F_COLS], f32, kind="ExternalInput")
    out_d = nc.dram_tensor("out", [SH, DK], f32, kind="ExternalOutput")

    with tile.TileContext(nc) as tc, ExitStack() as ctx:
        singles = ctx.enter_context(tc.tile_pool(name="singles", bufs=1))
        psA = ctx.enter_context(tc.tile_pool(name="psA", bufs=1, space="PSUM"))
        psB = ctx.enter_context(tc.tile_pool(name="psB", bufs=2, space="PSUM"))
        epool = ctx.enter_context(tc.tile_pool(name="epool", bufs=4))

        # ---------------- packed constant loads ----------------
        cbw_sb = singles.tile([128, BFW_COLS], bf16)
        nc.sync.dma_start(out=cbw_sb, in_=cbw_d[:, :])
        cf_sb = singles.tile([128, F_COLS], f32)
        nc.sync.dma_start(out=cf_sb, in_=cf_d[:, :])
        cbr_sb = singles.tile([128, BFR_COLS], bf16)

        wkv_sb = cbw_sb[:, BF_WKV:BF_ID].rearrange("p (d c) -> p d c", d=ND)
        ident_sb = cbw_sb[:, BF_ID:BF_ID + 128]
        wq_sb = cbr_sb[:, BFR_WQ:BFR_MASK].rearrange("p (d c) -> p d c", d=ND)
        mask_sb = cbr_sb[:, BFR_MASK:BFR_COLS].rearrange(
            "p (kb q) -> p kb q", kb=NCORES)
        bkv_sb = cf_sb[:, 0:1]
        bq_sb = cf_sb[0:DK, 1:2]
        identf_sb = cf_sb[:, 2:2 + 128]

        xT_sb = singles.tile([128, ND, S], bf16)
        xq_sb = singles.tile([128, ND, SH], bf16)
        kT_sb = singles.tile([DK, S], bf16)
        vt_sb = singles.tile([128, NB, DK + 1], bf16)
        qT_sb = singles.tile([DK, SH], bf16)
        # ones column of V~ (denominator accumulator)
        nc.scalar.activation(vt_sb[:, :, DK:DK + 1], ident_sb[:, 0:NB],
                             AF.Identity, bias=1.0, scale=0.0)

        def load_cols(c0, c1):
            cs = slice(c0, c1)
            nc.sync.dma_start(
                out=xT_sb[:, :, cs],
                in_=xT_d[:, cs].rearrange("(d p) s -> p d s", p=128))

        def one_pass(rep):
            load_cols(0, 256)
            load_cols(256, 512)
            nc.sync.dma_start(out=xq_sb, in_=xqT_d[:, :].rearrange(
                "(d p) s -> p d s", p=128))
            nc.sync.dma_start(out=cbr_sb, in_=cbr_d[:, :])
            for g in range(1, NCH):
                load_cols(CHUNK * g, CHUNK * (g + 1))

            av_started = [False] * SLOTS
            av_ps = None

            for g in range(NCH):
                cs0 = slice(CHUNK * g, CHUNK * g + 256)
                cs1 = slice(CHUNK * g + 256, CHUNK * (g + 1))
                # two interleaved 256-col chains in ONE open accumulation
                # group (start on the very first matmul, stop on the last;
                # per-element has_written handles overwrite-vs-accumulate)
                kv_ps = psB.tile([128, 2, 256], f32, tag="kvps", bufs=2,
                                 name="kv_ps")
                for d in range(ND):
                    nc.tensor.matmul(kv_ps[:, 0, :], lhsT=wkv_sb[:, d, :],
                                     rhs=xT_sb[:, d, cs0],
                                     start=(d == 0), stop=False,
                                     skip_group_check=True)
                    nc.tensor.matmul(kv_ps[:, 1, :], lhsT=wkv_sb[:, d, :],
                                     rhs=xT_sb[:, d, cs1],
                                     start=False, stop=(d == ND - 1),
                                     skip_group_check=True)
                cs = slice(CHUNK * g, CHUNK * (g + 1))
                nc.vector.tensor_scalar_add(
                    kT_sb[:, cs].rearrange("k (h s) -> k h s", h=2),
                    kv_ps[0:DK, :, :], bkv_sb[0:DK, 0:1])
                vT_h = epool.tile([DK, CHUNK], f32, tag="vth", name="vT_h")
                nc.vector.tensor_scalar_add(
                    vT_h.rearrange("k (h s) -> k h s", h=2),
                    kv_ps[DK:128, :, :], bkv_sb[DK:128, 0:1])
                t_ps = psB.tile([128, 4, QB], f32, tag="scps", bufs=3,
                                name="t_ps")
                for sl in range(4):
                    nc.tensor.transpose(t_ps[:, sl, 0:DK],
                                        vT_h[:, QB * sl:QB * (sl + 1)],
                                        identf_sb[0:DK, 0:DK])
                nc.scalar.copy(vt_sb[:, 4 * g:4 * (g + 1), 0:DK],
                               t_ps[:, :, 0:DK])

                if g == 0:
                    # Q^T for own 512 rows (after chunk-0 proj in PE order)
                    q_ps = psA.tile([DK, SH], f32, name="q_ps", tag="qps")
                    for d in range(ND):
                        nc.tensor.matmul(q_ps, lhsT=wq_sb[:, d, :],
                                         rhs=xq_sb[:, d, :],
                                         start=(d == 0), stop=(d == ND - 1))
                    nc.scalar.activation(qT_sb, q_ps, AF.Identity,
                                         bias=bq_sb[:, 0:1], scale=1.0)
                    av_acc = [singles.tile([DK + 1, QB], f32,
                                            name=f"av_acc{j}")
                              for j in range(SLOTS)]

                # attention: every q-slot whose causal prefix includes chunk g
                slots = [j for j in range(SLOTS) if g <= 2 * j + 1]
                parts = {}
                for j in slots:
                    qc = slice(QB * j, QB * (j + 1))
                    sc_ps = psB.tile([128, 4, QB], f32, tag="scps",
                                     bufs=3, name="sc_ps")
                    e_sb = epool.tile([128, 4, QB], bf16, tag=f"e{j % 2}",
                                      name="e_sb")
                    for sl in range(4):
                        kb = 4 * g + sl
                        nc.tensor.matmul(sc_ps[:, sl, :],
                                         lhsT=kT_sb[:, QB * kb:QB * (kb + 1)],
                                         rhs=qT_sb[:, qc],
                                         start=True, stop=True)
                    nc.scalar.activation(e_sb, sc_ps, AF.Exp, scale=0.125)
                    if g >= 2 * j:  # diagonal band: mask (ones/tri/zeros by c)
                        mi = 4 * (g - 2 * j)
                        nc.vector.tensor_mul(e_sb, e_sb,
                                             mask_sb[:, mi:mi + 4, :])
                    parts[j] = (qc, e_sb)
                # AV as one CLOSED accumulation group per (chunk, slot) in a
                # rotating scratch bank, then accumulated into SBUF on DVE:
                # one open group per bank at all times.
                for j in slots:
                    qc, e_sb = parts[j]
                    avp = psB.tile([DK + 1, QB], f32, tag="avp", bufs=2,
                                   name="avp")
                    for sl in range(4):
                        kb = 4 * g + sl
                        nc.tensor.matmul(avp, lhsT=vt_sb[:, kb, :],
                                         rhs=e_sb[:, sl, :],
                                         start=(sl == 0), stop=(sl == 3),
                                         skip_group_check=True)
                    if not av_started[j]:
                        nc.vector.tensor_copy(av_acc[j], avp)
                        av_started[j] = True
                    else:
                        nc.vector.tensor_tensor(av_acc[j], av_acc[j], avp,
                                                op=mybir.AluOpType.add)

                # epilogue per finished slot: slot j's accumulation completes
                # at chunk 2j+1 -> normalize and store it while later chunks
                # stream (only slot 3 remains in the tail)
                if g % 2 == 1:
                    j = g // 2
                    qc = slice(QB * j, QB * (j + 1))
                    t2 = psB.tile([128, 4, QB], f32, tag="scps", bufs=3,
                                  name="t2")
                    nc.tensor.transpose(t2[:, 0, 0:DK + 1], av_acc[j],
                                        identf_sb[0:DK + 1, 0:DK + 1])
                    rec = epool.tile([128, 1], f32, tag="rec", name="rec")
                    nc.vector.reciprocal(rec, t2[:, 0, DK:DK + 1])
                    out_sb = epool.tile([128, DK], f32, tag="osb",
                                        name="out_sb")
                    nc.vector.tensor_scalar_mul(out_sb, t2[:, 0, 0:DK], rec)
                    nc.sync.dma_start(out=out_d[QB * j:QB * (j + 1), :],
                                      in_=out_sb)

        for _rep in range(AMP):
            one_pass(_rep)

    nc.finalize()
    return nc


def _in_maps(x, Wq, bq, Wk, bk, Wv, bv):
    bf = ml_dtypes.bfloat16
    xT = np.ascontiguousarray(x.T).astype(bf)                      # [1024, 4096]
    tri = np.triu(np.ones((QB, QB), dtype=np.float32))  # E^T[k,q] valid iff k<=q

    # bf16 constant blobs
    wkvT = np.concatenate([Wk.T, Wv.T], axis=1)                    # [1024, 128]
    wkv_p = wkvT.reshape(ND, 128, 2 * DK).transpose(1, 0, 2).reshape(128, -1)
    wqT = Wq.T                                                     # [1024, 64]
    wq_p = wqT.reshape(ND, 128, DK).transpose(1, 0, 2).reshape(128, -1)
    ident = np.eye(128, dtype=np.float32)
    cbw = np.ascontiguousarray(
        np.concatenate([wkv_p, ident], axis=1).astype(bf))
    assert cbw.shape == (128, BFW_COLS)

    # f32 constant blob [128, F_COLS]
    cf = np.zeros((128, F_COLS), dtype=np.float32)
    cf[:, 0] = np.concatenate([bk, bv])
    cf[0:DK, 1] = bq
    cf[:, 2:2 + 128] = ident

    maps = []
    for c in range(NCORES):
        rows = np.concatenate([np.arange(QB * (8 * sl + c),
                                         QB * (8 * sl + c) + QB)
                               for sl in range(SLOTS)])
        xqT = np.ascontiguousarray(x[rows].T).astype(bf)           # [1024, 512]
        # diagonal-band mask: key index k within band vs own position c
        m = np.zeros((NCORES, QB, QB), dtype=np.float32)
        m[:c] = 1.0
        m[c] = tri
        mask_p = m.transpose(1, 0, 2).reshape(128, -1)
        cbr = np.ascontiguousarray(
            np.concatenate([wq_p, mask_p], axis=1).astype(bf))
        assert cbr.shape == (128, BFR_COLS)
        maps.append({"xT": xT, "xqT": xqT, "cbw": cbw, "cbr": cbr,
                     "cf": cf})
    return maps


def kernel(**inputs):
    global LAST_EXEC_NS
    x = np.asarray(inputs["x"], dtype=np.float32)
    args = [np.asarray(inputs[k], dtype=np.float32)
            for k in ("Wq", "bq", "Wk", "bk", "Wv", "bv")]
    in_maps = _in_maps(x, args[0], args[1], args[2], args[3], args[4], args[5])

    nc = _build_nc()
    from concourse.bass_utils import run_bass_kernel_spmd
    res = run_bass_kernel_spmd(nc, in_maps, core_ids=list(range(NCORES)))
    LAST_EXEC_NS = res.exec_time_ns

    out = np.zeros((S, DK), dtype=np.float32)
    for c in range(NCORES):
        r = res.results[c]["out"]
        for sl in range(SLOTS):
            b = 8 * sl + c
            out[QB * b:QB * (b + 1)] = r[QB * sl:QB * (sl + 1)]
    return out


# revision 33
# speedup vs baseline: 1.0108x; 1.0108x over previous
"""Causal single-head attention on 8 Trainium2 NeuronCores — fully local.

Problem: x[4096,1024] -> Q,K,V = x@W.T+b (d_k=64), out = softmax(causal(QK^T/8)) @ V.

Strategy (replicated K/V, zero communication):
  - Every core loads the FULL x^T in bf16 (8 MB) and computes K^T and V for
    all 4096 rows locally; no collective, no cross-core sync of any kind.
    The 8 MB stream overlaps the projection/attention pipeline.
  - Query blocks of 128 rows; core c owns global blocks {c, 8+c, 16+c, 24+c}
    (strided) -> every core runs the IDENTICAL program. Slot j attends key
    blocks 0..8j+7 (uniform); within the diagonal band (blocks 8j..8j+7) a
    per-core host-built mask (ones/tri/zeros by key index vs c) enforces
    exact causality. Off-band blocks are always fully valid -> no masking.
  - x-column chunks of 512 stream in; chunk g yields key blocks 4g..4g+3.
    K^T/V^T come from a stacked [Wk|Wv] projection run as TWO interleaved
    256-column PSUM chains (keeps the PE p-state ramp hot); V~ blocks are PE
    transposes of V^T with a ones-column appended so the AV matmul also
    accumulates the softmax denominator.
  - Attention (scores -> exp -> mask -> AV) for every (q-slot, key-group)
    pair runs as soon as its chunk is projected; AV matmuls are emitted
    round-robin across slots so accumulation chains interleave on the PE.
  - All attention matmuls in bf16 (rate-1 at any moving width); exp on
    ScalarE with the 1/8 scale folded in; accumulation in f32 PSUM.
  - Constants ride in packed blobs to amortize per-DMA overhead; the x
    stream is ordered chunk0 -> xq -> chunks 1-7 so the projection pipeline
    starts as early as possible.

  PSUM discipline (hardware-verified): a PSUM bank supports ONE open
  accumulation group at a time. Interleaving two start/stop chains in one
  bank silently corrupts accumulation on hardware (the simulator does not
  model this). Hence: the kv projection's two interleaved column chains
  share a single group (start on the very first matmul, stop on the last,
  per-element has_written handles overwrite-vs-accumulate); each (chunk,
  slot) AV block runs as a CLOSED group (start..stop) in a rotating scratch
  bank and is accumulated into per-slot SBUF tiles on VectorE.
"""

import os
import numpy as np
import ml_dtypes
from contextlib import ExitStack

S, DM, DK = 4096, 1024, 64
NCORES = 8
QB = 128                      # rows per block
SLOTS = 4                     # q-blocks per core
SH = QB * SLOTS               # 512 own query rows per core
NB = S // QB                  # 32 key blocks
CHUNK = 512                   # x columns per streamed chunk
NCH = S // CHUNK              # 8 chunks
ND = DM // 128                # 8 contraction chunks

# cb_w bf16 blob (early): wkv [8, 128] | ident [128]
BF_WKV = 0
BF_ID = ND * 128
BFW_COLS = BF_ID + 128
# cb_r bf16 blob (later): wq [8, 64] | mask [8, 128]
BFR_WQ = 0
BFR_MASK = ND * DK
BFR_COLS = BFR_MASK + NCORES * QB
# f32 blob layout: bkv [1] | bq [1] | identf [128]
F_COLS = 2 + 128

AMP = int(os.environ.get("KERNEL_AMP", "1"))  # repeat whole pipeline in-NEFF

LAST_EXEC_NS = None


def _build_nc():
    import concourse.bass as bass
    import concourse.bacc as bacc
    import concourse.mybir as mybir
    import concourse.tile as tile

    f32 = mybir.dt.float32
    bf16 = mybir.dt.bfloat16
    AF = mybir.ActivationFunctionType

    nc = bacc.Bacc(None, num_devices=NCORES)

    xT_d = nc.dram_tensor("xT", [DM, S], bf16, kind="ExternalInput")
    xqT_d = nc.dram_tensor("xqT", [DM, SH], bf16, kind="ExternalInput")
    cbw_d = nc.dram_tensor("cbw", [128, BFW_COLS], bf16, kind="ExternalInput")
    cbr_d = nc.dram_tensor("cbr", [128, BFR_COLS], bf16, kind="ExternalInput")
    cf_d = nc.dram_tensor("cf", [128, F_COLS], f32, kind="ExternalInput")
    out_d = nc.dram_tensor("out", [SH, DK], f32, kind="ExternalOutput")

    with tile.TileContext(nc) as tc, ExitStack() as ctx:
        singles = ctx.enter_context(tc.tile_pool(name="singles", bufs=1))
        psA = ctx.enter_context(tc.tile_pool(name="psA", bufs=1, space="PSUM"))
        psB = ctx.enter_context(tc.tile_pool(name="psB", bufs=2, space="PSUM"))
        epool = ctx.enter_context(tc.tile_pool(name="epool", bufs=4))

        # ---------------- packed constant loads ----------------
        cbw_sb = singles.tile([128, BFW_COLS], bf16)
        nc.sync.dma_start(out=cbw_sb, in_=cbw_d[:, :])
        cf_sb = singles.tile([128, F_COLS], f32)
        nc.sync.dma_start(out=cf_sb, in_=cf_d[:, :])
        cbr_sb = singles.tile([128, BFR_COLS], bf16)

        wkv_sb = cbw_sb[:, BF_WKV:BF_ID].rearrange("p (d c) -> p d c", d=ND)
        ident_sb = cbw_sb[:, BF_ID:BF_ID + 128]
        wq_sb = cbr_sb[:, BFR_WQ:BFR_MASK].rearrange("p (d c) -> p d c", d=ND)
        mask_sb = cbr_sb[:, BFR_MASK:BFR_COLS].rearrange(
            "p (kb q) -> p kb q", kb=NCORES)
        bkv_sb = cf_sb[:, 0:1]
        bq_sb = cf_sb[0:DK, 1:2]
        identf_sb = cf_sb[:, 2:2 + 128]

        xT_sb = singles.tile([128, ND, S], bf16)
        xq_sb = singles.tile([128, ND, SH], bf16)
        kT_sb = singles.tile([DK, S], bf16)
        vt_sb = singles.tile([128, NB, DK + 1], bf16)
        qT_sb = singles.tile([DK, SH], bf16)
        # ones column of V~ (denominator accumulator)
        nc.scalar.activation(vt_sb[:, :, DK:DK + 1], ident_sb[:, 0:NB],
                             AF.Identity, bias=1.0, scale=0.0)

        def load_cols(c0, c1):
            cs = slice(c0, c1)
            nc.sync.dma_start(
                out=xT_sb[:, :, cs],
                in_=xT_d[:, cs].rearrange("(d p) s -> p d s", p=128))

        def one_pass(rep):
            load_cols(0, 256)
            load_cols(256, 512)
            nc.sync.dma_start(out=xq_sb, in_=xqT_d[:, :].rearrange(
                "(d p) s -> p d s", p=128))
            nc.sync.dma_start(out=cbr_sb, in_=cbr_d[:, :])
            for g in range(1, NCH - 1):
                load_cols(CHUNK * g, CHUNK * (g + 1))
            load_cols(S - 512, S - 256)
            load_cols(S - 256, S)

            av_started = [False] * SLOTS
            av_acc = [singles.tile([DK + 1, QB], f32, name=f"av_acc{j}")
                      for j in range(SLOTS)]
            vT_hs = {}

            def emit_proj(c0, W=CHUNK):
                h = W // 2
                cs0 = slice(c0, c0 + h)
                cs1 = slice(c0 + h, c0 + W)
                # two interleaved 256-col chains in ONE open accumulation
                # group (start on the very first matmul, stop on the last;
                # per-element has_written handles overwrite-vs-accumulate)
                kv_ps = psB.tile([128, 2, 256], f32, tag="kvps", bufs=2,
                                 name="kv_ps")
                kv0, kv1 = kv_ps[:, 0, 0:h], kv_ps[:, 1, 0:h]
                for d in range(ND):
                    nc.tensor.matmul(kv0, lhsT=wkv_sb[:, d, :],
                                     rhs=xT_sb[:, d, cs0],
                                     start=(d == 0), stop=False,
                                     skip_group_check=True)
                    nc.tensor.matmul(kv1, lhsT=wkv_sb[:, d, :],
                                     rhs=xT_sb[:, d, cs1],
                                     start=False, stop=(d == ND - 1),
                                     skip_group_check=True)
                cs = slice(c0, c0 + W)
                nc.vector.tensor_scalar_add(
                    kT_sb[:, cs].rearrange("k (h s) -> k h s", h=2),
                    kv_ps[0:DK, :, 0:h], bkv_sb[0:DK, 0:1])
                vT_h = epool.tile([DK, CHUNK], f32, tag="vth", name="vT_h")
                nc.vector.tensor_scalar_add(
                    vT_h[:, 0:W].rearrange("k (h s) -> k h s", h=2),
                    kv_ps[DK:128, :, 0:h], bkv_sb[DK:128, 0:1])
                t_ps = psB.tile([128, 4, QB], f32, tag="scps", bufs=3,
                                name="t_ps")
                for sl in range(W // QB):
                    nc.tensor.transpose(t_ps[:, sl, 0:DK],
                                        vT_h[:, QB * sl:QB * (sl + 1)],
                                        identf_sb[0:DK, 0:DK])
                nc.scalar.copy(vt_sb[:, c0 // QB:(c0 + W) // QB, 0:DK],
                               t_ps[:, 0:W // QB, 0:DK])

            def emit_attn(c0, W=CHUNK):
                kb0, nb = c0 // QB, W // QB
                # attention: every q-slot whose causal prefix includes them
                slots = [j for j in range(SLOTS) if kb0 < 8 * j + 8]
                parts = {}
                for j in slots:
                    qc = slice(QB * j, QB * (j + 1))
                    sc_ps = psB.tile([128, 4, QB], f32, tag="scps",
                                     bufs=3, name="sc_ps")
                    e_sb = epool.tile([128, 4, QB], bf16, tag=f"e{j % 2}",
                                      name="e_sb")
                    for sl in range(nb):
                        kb = kb0 + sl
                        nc.tensor.matmul(sc_ps[:, sl, :],
                                         lhsT=kT_sb[:, QB * kb:QB * (kb + 1)],
                                         rhs=qT_sb[:, qc],
                                         start=True, stop=True)
                    nc.scalar.activation(e_sb[:, 0:nb, :], sc_ps[:, 0:nb, :],
                                         AF.Exp, scale=0.125)
                    if kb0 + nb > 8 * j:  # diagonal band: ones/tri/zeros by c
                        mi = kb0 - 8 * j
                        nc.vector.tensor_mul(e_sb[:, 0:nb, :],
                                             e_sb[:, 0:nb, :],
                                             mask_sb[:, mi:mi + nb, :])
                    parts[j] = (qc, e_sb)
                # AV as one CLOSED accumulation group per (chunk, slot) in a
                # rotating scratch bank, then accumulated into SBUF on DVE:
                # one open group per bank at all times.
                for j in slots:
                    qc, e_sb = parts[j]
                    avp = psB.tile([DK + 1, QB], f32, tag="avp", bufs=2,
                                   name="avp")
                    for sl in range(nb):
                        kb = kb0 + sl
                        nc.tensor.matmul(avp, lhsT=vt_sb[:, kb, :],
                                         rhs=e_sb[:, sl, :],
                                         start=(sl == 0), stop=(sl == nb - 1),
                                         skip_group_check=True)
                    if not av_started[j]:
                        nc.vector.tensor_copy(av_acc[j], avp)
                        av_started[j] = True
                    else:
                        nc.vector.tensor_tensor(av_acc[j], av_acc[j], avp,
                                                op=mybir.AluOpType.add)

                # epilogue per finished slot: normalize and store slot j as
                # soon as its last key block (8j+7) is accumulated
                if (kb0 + nb) % 8 == 0 and (kb0 + nb) // 8 - 1 < SLOTS:
                    j = (kb0 + nb) // 8 - 1
                    t2 = psB.tile([128, 4, QB], f32, tag="scps", bufs=3,
                                  name="t2")
                    nc.tensor.transpose(t2[:, 0, 0:DK + 1], av_acc[j],
                                        identf_sb[0:DK + 1, 0:DK + 1])
                    rec = epool.tile([128, 1], f32, tag="rec", name="rec")
                    nc.vector.reciprocal(rec, t2[:, 0, DK:DK + 1])
                    out_sb = epool.tile([128, DK], f32, tag="osb",
                                        name="out_sb")
                    nc.vector.tensor_scalar_mul(out_sb, t2[:, 0, 0:DK], rec)
                    nc.sync.dma_start(out=out_d[QB * j:QB * (j + 1), :],
                                      in_=out_sb)

            emit_proj(0, CHUNK)
            # Q^T for own 512 rows (after chunk-0 proj in PE order)
            q_ps = psA.tile([DK, SH], f32, name="q_ps", tag="qps")
            for d in range(ND):
                nc.tensor.matmul(q_ps, lhsT=wq_sb[:, d, :],
                                 rhs=xq_sb[:, d, :],
                                 start=(d == 0), stop=(d == ND - 1))
            nc.scalar.activation(qT_sb, q_ps, AF.Identity,
                                 bias=bq_sb[:, 0:1], scale=1.0)
            emit_attn(0, CHUNK)
            for g in range(1, NCH - 1):
                emit_proj(CHUNK * g)
                emit_attn(CHUNK * g)
            emit_proj(S - 512, 256)
            emit_attn(S - 512, 256)
            emit_proj(S - 256, 256)
            emit_attn(S - 256, 256)

        for _rep in range(AMP):
            one_pass(_rep)

    nc.finalize()
    return nc


def _in_maps(x, Wq, bq, Wk, bk, Wv, bv):
    bf = ml_dtypes.bfloat16
    xT = np.ascontiguousarray(x.T).astype(bf)                      # [1024, 4096]
    tri = np.triu(np.ones((QB, QB), dtype=np.float32))  # E^T[k,q] valid iff k<=q

    # bf16 constant blobs
    wkvT = np.concatenate([Wk.T, Wv.T], axis=1)                    # [1024, 128]
    wkv_p = wkvT.reshape(ND, 128, 2 * DK).transpose(1, 0, 2).reshape(128, -1)
    wqT = Wq.T                                                     # [1024, 64]
    wq_p = wqT.reshape(ND, 128, DK).transpose(1, 0, 2).reshape(128, -1)
    ident = np.eye(128, dtype=np.float32)
    cbw = np.ascontiguousarray(
        np.concatenate([wkv_p, ident], axis=1).astype(bf))
    assert cbw.shape == (128, BFW_COLS)

    # f32 constants, bit-packed into the bf16 blob
    cf = np.zeros((128, F_COLS), dtype=np.float32)
    cf[:, 0] = np.concatenate([bk, bv])
    cf[0:DK, 1] = bq
    cf[:, 2:2 + 128] = ident

    maps = []
    for c in range(NCORES):
        rows = np.concatenate([np.arange(QB * (8 * sl + c),
                                         QB * (8 * sl + c) + QB)
                               for sl in range(SLOTS)])
        xqT = np.ascontiguousarray(x[rows].T).astype(bf)           # [1024, 512]
        # diagonal-band mask: key index k within band vs own position c
        m = np.zeros((NCORES, QB, QB), dtype=np.float32)
        m[:c] = 1.0
        m[c] = tri
        mask_p = m.transpose(1, 0, 2).reshape(128, -1)
        cbr = np.ascontiguousarray(
            np.concatenate([wq_p, mask_p], axis=1).astype(bf))
        assert cbr.shape == (128, BFR_COLS)
        maps.append({"xT": xT, "xqT": xqT, "cbw": cbw, "cbr": cbr,
                     "cf": cf})
    return maps


def kernel(**inputs):
    global LAST_EXEC_NS
    x = np.asarray(inputs["x"], dtype=np.float32)
    args = [np.asarray(inputs[k], dtype=np.float32)
            for k in ("Wq", "bq", "Wk", "bk", "Wv", "bv")]
    in_maps = _in_maps(x, args[0], args[1], args[2], args[3], args[4], args[5])

    nc = _build_nc()
    from concourse.bass_utils import run_bass_kernel_spmd
    res = run_bass_kernel_spmd(nc, in_maps, core_ids=list(range(NCORES)))
    LAST_EXEC_NS = res.exec_time_ns

    out = np.zeros((S, DK), dtype=np.float32)
    for c in range(NCORES):
        r = res.results[c]["out"]
        for sl in range(SLOTS):
            b = 8 * sl + c
            out[QB * b:QB * (b + 1)] = r[QB * sl:QB * (sl + 1)]
    return out


# revision 40
# speedup vs baseline: 1.0248x; 1.0139x over previous
"""Causal single-head attention on 8 Trainium2 NeuronCores — fully local.

Problem: x[4096,1024] -> Q,K,V = x@W.T+b (d_k=64), out = softmax(causal(QK^T/8)) @ V.

Strategy (replicated K/V, zero communication):
  - Every core loads the FULL x^T in bf16 (8 MB) and computes K^T and V for
    all 4096 rows locally; no collective, no cross-core sync of any kind.
    The 8 MB stream overlaps the projection/attention pipeline.
  - Query blocks of 128 rows; core c owns global blocks {c, 8+c, 16+c, 24+c}
    (strided) -> every core runs the IDENTICAL program. Slot j attends key
    blocks 0..8j+7 (uniform); within the diagonal band (blocks 8j..8j+7) a
    per-core host-built mask (ones/tri/zeros by key index vs c) enforces
    exact causality. Off-band blocks are always fully valid -> no masking.
  - x-column chunks of 512 stream in; chunk g yields key blocks 4g..4g+3.
    K^T/V^T come from a stacked [Wk|Wv] projection run as TWO interleaved
    256-column PSUM chains (keeps the PE p-state ramp hot); V~ blocks are PE
    transposes of V^T with a ones-column appended so the AV matmul also
    accumulates the softmax denominator.
  - Attention (scores -> exp -> mask -> AV) for every (q-slot, key-group)
    pair runs as soon as its chunk is projected; AV matmuls are emitted
    round-robin across slots so accumulation chains interleave on the PE.
  - All attention matmuls in bf16 (rate-1 at any moving width); exp on
    ScalarE with the 1/8 scale folded in; accumulation in f32 PSUM.
  - Constants ride in packed blobs to amortize per-DMA overhead; the x
    stream is ordered chunk0 -> xq -> chunks 1-7 so the projection pipeline
    starts as early as possible.

  PSUM discipline (hardware-verified): a PSUM bank supports ONE open
  accumulation group at a time. Interleaving two start/stop chains in one
  bank silently corrupts accumulation on hardware (the simulator does not
  model this). Hence: the kv projection's two interleaved column chains
  share a single group (start on the very first matmul, stop on the last,
  per-element has_written handles overwrite-vs-accumulate); each (chunk,
  slot) AV block runs as a CLOSED group (start..stop) in a rotating scratch
  bank and is accumulated into per-slot SBUF tiles on VectorE.
"""

import os
import numpy as np
import ml_dtypes
from contextlib import ExitStack

S, DM, DK = 4096, 1024, 64
NCORES = 8
QB = 128                      # rows per block
SLOTS = 4                     # q-blocks per core
SH = QB * SLOTS               # 512 own query rows per core
NB = S // QB                  # 32 key blocks
CHUNK = 512                   # x columns per streamed chunk
NCH = S // CHUNK              # 8 chunks
ND = DM // 128                # 8 contraction chunks

# cb_w bf16 blob (early): wkv [8, 128] | ident [128]
BF_WKV = 0
BF_ID = ND * 128
BFW_COLS = BF_ID + 128
# cb_r bf16 blob (later): wq [8, 64] | mask [8, 128]
BFR_WQ = 0
BFR_MASK = ND * DK
BFR_COLS = BFR_MASK + NCORES * QB
# f32 blob layout: bkv [1] | bq [1] | identf [128]
F_COLS = 2 + 128

AMP = int(os.environ.get("KERNEL_AMP", "1"))  # repeat whole pipeline in-NEFF

LAST_EXEC_NS = None


def _build_nc():
    import concourse.bass as bass
    import concourse.bacc as bacc
    import concourse.mybir as mybir
    import concourse.tile as tile

    f32 = mybir.dt.float32
    bf16 = mybir.dt.bfloat16
    AF = mybir.ActivationFunctionType

    nc = bacc.Bacc(None, num_devices=NCORES)

    xT_d = nc.dram_tensor("xT", [DM, S], bf16, kind="ExternalInput")
    xqT_d = nc.dram_tensor("xqT", [DM, SH], bf16, kind="ExternalInput")
    cbw_d = nc.dram_tensor("cbw", [128, BFW_COLS], bf16, kind="ExternalInput")
    cbr_d = nc.dram_tensor("cbr", [128, BFR_COLS], bf16, kind="ExternalInput")
    cf_d = nc.dram_tensor("cf", [128, F_COLS], f32, kind="ExternalInput")
    out_d = nc.dram_tensor("out", [SH, DK], f32, kind="ExternalOutput")

    with tile.TileContext(nc) as tc, ExitStack() as ctx:
        singles = ctx.enter_context(tc.tile_pool(name="singles", bufs=1))
        psA = ctx.enter_context(tc.tile_pool(name="psA", bufs=1, space="PSUM"))
        psB = ctx.enter_context(tc.tile_pool(name="psB", bufs=2, space="PSUM"))
        epool = ctx.enter_context(tc.tile_pool(name="epool", bufs=4))

        # ---------------- packed constant loads ----------------
        cbw_sb = singles.tile([128, BFW_COLS], bf16)
        nc.sync.dma_start(out=cbw_sb, in_=cbw_d[:, :])
        cf_sb = singles.tile([128, F_COLS], f32)
        nc.sync.dma_start(out=cf_sb, in_=cf_d[:, :])
        cbr_sb = singles.tile([128, BFR_COLS], bf16)

        wkv_sb = cbw_sb[:, BF_WKV:BF_ID].rearrange("p (d c) -> p d c", d=ND)
        ident_sb = cbw_sb[:, BF_ID:BF_ID + 128]
        wq_sb = cbr_sb[:, BFR_WQ:BFR_MASK].rearrange("p (d c) -> p d c", d=ND)
        mask_sb = cbr_sb[:, BFR_MASK:BFR_COLS].rearrange(
            "p (kb q) -> p kb q", kb=NCORES)
        bkv_sb = cf_sb[:, 0:1]
        bq_sb = cf_sb[0:DK, 1:2]
        identf_sb = cf_sb[:, 2:2 + 128]

        xT_sb = singles.tile([128, ND, S], bf16)
        xq_sb = singles.tile([128, ND, SH], bf16)
        kT_sb = singles.tile([DK, S], bf16)
        vt_sb = singles.tile([128, NB, DK + 1], bf16)
        qT_sb = singles.tile([DK, SH], bf16)
        # ones column of V~ (denominator accumulator)
        nc.scalar.activation(vt_sb[:, :, DK:DK + 1], ident_sb[:, 0:NB],
                             AF.Identity, bias=1.0, scale=0.0)

        def load_cols(c0, c1):
            cs = slice(c0, c1)
            nc.sync.dma_start(
                out=xT_sb[:, :, cs],
                in_=xT_d[:, cs].rearrange("(d p) s -> p d s", p=128))

        def one_pass(rep):
            load_cols(0, 256)
            load_cols(256, 512)
            nc.sync.dma_start(out=xq_sb, in_=xqT_d[:, :].rearrange(
                "(d p) s -> p d s", p=128))
            nc.sync.dma_start(out=cbr_sb, in_=cbr_d[:, :])
            for g in range(1, NCH - 1):
                load_cols(CHUNK * g, CHUNK * (g + 1))
            load_cols(S - 512, S - 256)
            load_cols(S - 256, S)

            av_started = [False] * SLOTS
            av_acc = [singles.tile([DK + 1, QB], f32, name=f"av_acc{j}")
                      for j in range(SLOTS)]

            def emit_proj(c0, W=CHUNK):
                h = W // 2
                cs0 = slice(c0, c0 + h)
                cs1 = slice(c0 + h, c0 + W)
                # two interleaved 256-col chains in ONE open accumulation
                # group (start on the very first matmul, stop on the last;
                # per-element has_written handles overwrite-vs-accumulate)
                kv_ps = psB.tile([128, 2, 256], f32, tag="kvps", bufs=2,
                                 name="kv_ps")
                kv0, kv1 = kv_ps[:, 0, 0:h], kv_ps[:, 1, 0:h]
                for d in range(ND):
                    nc.tensor.matmul(kv0, lhsT=wkv_sb[:, d, :],
                                     rhs=xT_sb[:, d, cs0],
                                     start=(d == 0), stop=False,
                                     skip_group_check=True)
                    nc.tensor.matmul(kv1, lhsT=wkv_sb[:, d, :],
                                     rhs=xT_sb[:, d, cs1],
                                     start=False, stop=(d == ND - 1),
                                     skip_group_check=True)
                cs = slice(c0, c0 + W)
                nc.vector.tensor_scalar_add(
                    kT_sb[:, cs].rearrange("k (h s) -> k h s", h=2),
                    kv_ps[0:DK, :, 0:h], bkv_sb[0:DK, 0:1])
                vT_h = epool.tile([DK, CHUNK], f32, tag="vth", name="vT_h")
                nc.vector.tensor_scalar_add(
                    vT_h[:, 0:W].rearrange("k (h s) -> k h s", h=2),
                    kv_ps[DK:128, :, 0:h], bkv_sb[DK:128, 0:1])
                t_ps = psB.tile([128, 4, QB], f32, tag="scps", bufs=3,
                                name="t_ps")
                for sl in range(W // QB):
                    nc.tensor.transpose(t_ps[:, sl, 0:DK],
                                        vT_h[:, QB * sl:QB * (sl + 1)],
                                        identf_sb[0:DK, 0:DK])
                nc.scalar.copy(vt_sb[:, c0 // QB:(c0 + W) // QB, 0:DK],
                               t_ps[:, 0:W // QB, 0:DK])

            def emit_attn(c0, W=CHUNK):
                kb0, nb = c0 // QB, W // QB
                # attention: every q-slot whose causal prefix includes them
                slots = [j for j in range(SLOTS) if kb0 < 8 * j + 8]
                parts = {}
                for j in slots:
                    qc = slice(QB * j, QB * (j + 1))
                    sc_ps = psB.tile([128, 4, QB], f32, tag="scps",
                                     bufs=3, name="sc_ps")
                    e_sb = epool.tile([128, 4, QB], bf16, tag=f"e{j % 2}",
                                      name="e_sb")
                    for sl in range(nb):
                        kb = kb0 + sl
                        nc.tensor.matmul(sc_ps[:, sl, :],
                                         lhsT=kT_sb[:, QB * kb:QB * (kb + 1)],
                                         rhs=qT_sb[:, qc],
                                         start=True, stop=True)
                    nc.scalar.activation(e_sb[:, 0:nb, :], sc_ps[:, 0:nb, :],
                                         AF.Exp, scale=0.125)
                    if kb0 + nb > 8 * j:  # diagonal band: ones/tri/zeros by c
                        mi = kb0 - 8 * j
                        nc.vector.tensor_mul(e_sb[:, 0:nb, :],
                                             e_sb[:, 0:nb, :],
                                             mask_sb[:, mi:mi + nb, :])
                    parts[j] = (qc, e_sb)
                # AV as one CLOSED accumulation group per (chunk, slot) in a
                # rotating scratch bank, then accumulated into SBUF on DVE:
                # one open group per bank at all times.
                for j in slots:
                    qc, e_sb = parts[j]
                    avp = psB.tile([DK + 1, QB], f32, tag="avp", bufs=2,
                                   name="avp")
                    for sl in range(nb):
                        kb = kb0 + sl
                        nc.tensor.matmul(avp, lhsT=vt_sb[:, kb, :],
                                         rhs=e_sb[:, sl, :],
                                         start=(sl == 0), stop=(sl == nb - 1),
                                         skip_group_check=True)
                    if not av_started[j]:
                        nc.vector.tensor_copy(av_acc[j], avp)
                        av_started[j] = True
                    else:
                        nc.vector.tensor_tensor(av_acc[j], av_acc[j], avp,
                                                op=mybir.AluOpType.add)

                # epilogue per finished slot: normalize and store slot j as
                # soon as its last key block (8j+7) is accumulated
                if (kb0 + nb) % 8 == 0 and (kb0 + nb) // 8 - 1 < SLOTS:
                    j = (kb0 + nb) // 8 - 1
                    t2 = psB.tile([128, 4, QB], f32, tag="scps", bufs=3,
                                  name="t2")
                    nc.tensor.transpose(t2[:, 0, 0:DK + 1], av_acc[j],
                                        identf_sb[0:DK + 1, 0:DK + 1])
                    rec = epool.tile([128, 1], f32, tag="rec", name="rec")
                    nc.vector.reciprocal(rec, t2[:, 0, DK:DK + 1])
                    out_sb = epool.tile([128, DK], f32, tag="osb",
                                        name="out_sb")
                    nc.vector.tensor_scalar_mul(out_sb, t2[:, 0, 0:DK], rec)
                    nc.sync.dma_start(out=out_d[QB * j:QB * (j + 1), :],
                                      in_=out_sb)

            emit_proj(0, CHUNK)
            # Q^T for own 512 rows (after chunk-0 proj in PE order)
            q_ps = psA.tile([DK, SH], f32, name="q_ps", tag="qps")
            for d in range(ND):
                nc.tensor.matmul(q_ps, lhsT=wq_sb[:, d, :],
                                 rhs=xq_sb[:, d, :],
                                 start=(d == 0), stop=(d == ND - 1))
            nc.scalar.activation(qT_sb, q_ps, AF.Identity,
                                 bias=bq_sb[:, 0:1], scale=1.0)
            emit_attn(0, CHUNK)
            for g in range(1, NCH - 1):
                emit_proj(CHUNK * g)
                emit_attn(CHUNK * g)
            emit_proj(S - 512, 256)
            emit_attn(S - 512, 256)
            emit_proj(S - 256, 256)
            emit_attn(S - 256, 256)

        for _rep in range(AMP):
            one_pass(_rep)

    nc.finalize()
    return nc


def _in_maps(x, Wq, bq, Wk, bk, Wv, bv):
    bf = ml_dtypes.bfloat16
    xT = np.ascontiguousarray(x.T).astype(bf)                      # [1024, 4096]
    tri = np.triu(np.ones((QB, QB), dtype=np.float32))  # E^T[k,q] valid iff k<=q

    # bf16 constant blobs
    wkvT = np.concatenate([Wk.T, Wv.T], axis=1)                    # [1024, 128]
    wkv_p = wkvT.reshape(ND, 128, 2 * DK).transpose(1, 0, 2).reshape(128, -1)
    wqT = Wq.T                                                     # [1024, 64]
    wq_p = wqT.reshape(ND, 128, DK).transpose(1, 0, 2).reshape(128, -1)
    ident = np.eye(128, dtype=np.float32)
    cbw = np.ascontiguousarray(
        np.concatenate([wkv_p, ident], axis=1).astype(bf))
    assert cbw.shape == (128, BFW_COLS)

    # f32 constants, bit-packed into the bf16 blob
    cf = np.zeros((128, F_COLS), dtype=np.float32)
    cf[:, 0] = np.concatenate([bk, bv])
    cf[0:DK, 1] = bq
    cf[:, 2:2 + 128] = ident

    maps = []
    for c in range(NCORES):
        rows = np.concatenate([np.arange(QB * (8 * sl + c),
                                         QB * (8 * sl + c) + QB)
                               for sl in range(SLOTS)])
        xqT = np.ascontiguousarray(x[rows].T).astype(bf)           # [1024, 512]
        # diagonal-band mask: key index k within band vs own position c
        m = np.zeros((NCORES, QB, QB), dtype=np.float32)
        m[:c] = 1.0
        m[c] = tri
        mask_p = m.transpose(1, 0, 2).reshape(128, -1)
        cbr = np.ascontiguousarray(
            np.concatenate([wq_p, mask_p], axis=1).astype(bf))
        assert cbr.shape == (128, BFR_COLS)
        maps.append({"xT": xT, "xqT": xqT, "cbw": cbw, "cbr": cbr,
                     "cf": cf})
    return maps


def kernel(**inputs):
    global LAST_EXEC_NS
    x = np.asarray(inputs["x"], dtype=np.float32)
    args = [np.asarray(inputs[k], dtype=np.float32)
            for k in ("Wq", "bq", "Wk", "bk", "Wv", "bv")]
    in_maps = _in_maps(x, args[0], args[1], args[2], args[3], args[4], args[5])

    nc = _build_nc()
    from concourse.bass_utils import run_bass_kernel_spmd
    res = run_bass_kernel_spmd(nc, in_maps, core_ids=list(range(NCORES)))
    LAST_EXEC_NS = res.exec_time_ns

    out = np.zeros((S, DK), dtype=np.float32)
    for c in range(NCORES):
        r = res.results[c]["out"]
        for sl in range(SLOTS):
            b = 8 * sl + c
            out[QB * b:QB * (b + 1)] = r[QB * sl:QB * (sl + 1)]
    return out
